# revision 13
# baseline (speedup 1.0000x reference)
"""CARAFE upsampling (k=5, x2, C=256) as a Bass/Tile kernel on 8 NeuronCores.

Math (per output pixel):
  out[b, Y, X, c] = sum_{ky,kx} softmax(masks[b,Y,X,:])[ky*5+kx]
                    * feat[b, Y//2+ky-2, X//2+kx-2, c]       (zero padded)

Mapping: pure data parallel over (batch, 32-output-row strips) -> 8 cores.
Each core handles 16 output-row pairs; a row pair (Y=2y, 2y+1) contracts
over x' (64 source cols + 2 zero pad each side -> 68 partitions) with a
banded weight matrix per (row r, ky):
  lhsT[x'pad, X] = exp(mask[Y, X, ky*5 + (x'pad - X//2)])   (band, else 0)
so out_psum[X, c] = sum_ky lhsT_ky.T @ feat_row[y+ky-2]  (PSUM accumulate),
then scaled by 1/sum(exp) on PSUM->SBUF eviction.

The banded lhsT is built by one diagonal SBUF->SBUF DMA per row pair from
an exp'd, freely-reordered weight tile:
  e5[u, kx*20 + v*10 + r*5 + ky] = exp(masks[2y+r, 2u+v, ky*5+kx])
  AT[u+kx (part), (2u+v)*10 + r*5 + ky] = e5[u, ...]
dest AP [[1300,64],[1280,5],[1,20]] / src [[100,64],[20,5],[1,20]]
-> 320 descriptors of 80 contiguous bytes. Out-of-band entries stay zero
(memset once; the in-band region is fully rewritten every iteration), and
band rows touching the x' zero-pad multiply zeroed feature partitions.
"""

import sys

for _p in ("/opt/trn_rl_repo",):
    if _p not in sys.path:
        sys.path.insert(0, _p)

import numpy as np

B = 2
H_IN = 64
W_IN = 64
C = 256
H_OUT = 128
W_OUT = 128
KK = 25
N_CORES = 8
ROWS_PER_CORE = H_OUT * B // N_CORES  # 32 output rows
PAIRS = ROWS_PER_CORE // 2  # 16
SLAB = PAIRS + 4  # feature rows a core touches (16 + 2 pad each side)
XPAD = W_IN + 4  # 68 contraction partitions

_NC_CACHE = {}


def _build_nc(mm_dtype_name="float32"):
    import concourse.bacc as bacc
    import concourse.mybir as mybir
    from concourse import tile

    dt = mybir.dt
    f32 = dt.float32
    mm_dt = getattr(dt, mm_dtype_name)

    # detect_race_conditions=False: the sim's race shadow over-approximates
    # the diagonal scatter AP's footprint (flat element span read as a
    # per-partition byte span), falsely flagging disjoint tiles. Tile's own
    # dependency tracking handles the diagonal AP correctly (verified with
    # a micro-test: the scatter is sem-ordered against same-tensor
    # memset/producer/consumer accesses).
    nc = bacc.Bacc("TRN2", target_bir_lowering=False, debug=False,
                   num_devices=N_CORES, detect_race_conditions=False)
    feat = nc.dram_tensor("feat", [SLAB, W_IN, C], f32, kind="ExternalInput")
    masks = nc.dram_tensor("masks", [ROWS_PER_CORE, W_OUT, KK], f32,
                           kind="ExternalInput")
    out = nc.dram_tensor("out", [ROWS_PER_CORE, W_OUT, C], f32,
                         kind="ExternalOutput")

    AP = type(feat[:])

    NBUF = 3

    with tile.TileContext(nc) as tc:
        with (
            tc.tile_pool(name="big", bufs=1) as big,
            tc.tile_pool(name="psum", bufs=4, space="PSUM") as psumpool,
        ):
            # All loop tiles are explicit arrays (manual multi-buffering) so
            # that at_all can be created LAST: the sim's coarse span
            # bookkeeping for the diagonal scatter AP extends upward from
            # at_all's base, so nothing may live above it.
            def mk(shape, tag, n):
                return [big.tile(shape, f32, tag=f"{tag}{i}",
                                 name=f"{tag}{i}") for i in range(n)]

            feat_sb = big.tile([XPAD, SLAB * C], f32, tag="feat_sb",
                               name="feat_sb")
            m4s = mk([64, 100], "m4", NBUF)
            e5s = mk([64, 100], "e5", NBUF)
            m2s = mk([128, 50], "m2", NBUF)
            escr = mk([128, KK], "escr", 2)
            esums = mk([128, 2], "esum", NBUF)
            invsums = mk([128, 2], "invsum", NBUF)
            ots = mk([128, C], "ot", 4)
            # merged double-buffered banded-weights tensor, topmost in SBUF
            at_all = big.tile([XPAD, 2 * W_OUT * 10], f32, tag="at_all",
                              name="at_all")
            # DRAM staging for the banded scatter: the SBUF descriptor
            # generators (HWDGE and SWDGE) cannot walk mixed
            # partition+byte ("diagonal") steps, but DRAM is flat, so the
            # diagonal write goes to DRAM and a plain 2D load rebuilds the
            # banded SBUF tile.
            stage = nc.dram_tensor("at_stage", [2, XPAD, W_OUT * 10], f32,
                                   kind="Internal")

            nc.vector.memset(feat_sb[:], 0.0)
            nc.vector.memset(at_all[:], 0.0)
            # zero both staging buffers once (bands are fully rewritten
            # every iteration; out-of-band stays zero forever)
            for hb in range(2):
                nc.sync.dma_start(out=stage[hb], in_=at_all[:, :W_OUT * 10])

            # Feature rows: partition x'pad = x' + 2 (0,1,66,67 stay zero).
            nc.sync.dma_start(
                out=feat_sb[2:2 + W_IN, :].rearrange("x (r c) -> x r c",
                                                     r=SLAB),
                in_=feat[:].rearrange("r x c -> x r c"),
            )

            for j in range(PAIRS):
                at_off = (j % 2) * W_OUT * 10

                # masks for rows (2j, 2j+1): m4[u, r*50 + v*25 + p]
                m4 = m4s[j % NBUF]
                nc.sync.dma_start(
                    out=m4[:].rearrange("u (r v p) -> u r (v p)", r=2, v=2),
                    in_=masks[2 * j:2 * j + 2].rearrange(
                        "r (u v) p -> u r (v p)", v=2),
                )

                # exp into scatter-ready layout
                # e5[u, kx*20 + v*10 + r*5 + ky] = exp(m4[u, r*50+v*25+ky*5+kx])
                e5 = e5s[j % NBUF]
                m4v = m4[:].rearrange("u (r v ky kx) -> u r v kx ky",
                                      r=2, v=2, ky=5)
                e5v = e5[:].rearrange("u (kx v r ky) -> u r v kx ky",
                                      kx=5, v=2, r=2)
                for r in range(2):
                    for v in range(2):
                        nc.scalar.activation(
                            out=e5v[:, r, v],
                            in_=m4v[:, r, v],
                            func=mybir.ActivationFunctionType.Exp,
                        )

                # Softmax denominators in output-pixel layout: second load of
                # the same mask rows as [X=128, (r, p)], exp with accum_out
                # giving per-partition sums, then reciprocal.
                m2 = m2s[j % NBUF]
                nc.sync.dma_start(
                    out=m2[:].rearrange("x (r p) -> x r p", r=2),
                    in_=masks[2 * j:2 * j + 2].rearrange("r x p -> x r p"),
                )
                esum = esums[j % NBUF]
                for r in range(2):
                    nc.scalar.activation(
                        out=escr[r][:],
                        in_=m2[:, r * KK:(r + 1) * KK],
                        func=mybir.ActivationFunctionType.Exp,
                        accum_out=esum[:, r:r + 1],
                    )
                invsum = invsums[j % NBUF]
                nc.vector.reciprocal(invsum[:], esum[:])

                # Banded scatter via DRAM staging:
                #   stage[hb][u+kx, (2u+v)*10 + r*5 + ky] = e5[u, ...]
                hb = j % 2
                W10 = W_OUT * 10
                stage_full = stage[hb]
                dst = AP(tensor=stage_full.tensor,
                         offset=stage_full.offset,
                         ap=[[W10 + 20, 64], [W10, 5], [1, 20]])
                e5_full = e5[:]
                src = AP(tensor=e5_full.tensor, offset=e5_full.offset,
                         ap=[[100, 64], [20, 5], [1, 20]])
                nc.sync.dma_start(out=dst, in_=src)
                # load the banded tile back into SBUF
                nc.sync.dma_start(out=at_all[:, at_off:at_off + W10],
                                  in_=stage[hb])

                # 5 accumulating matmuls per output row
                for r in range(2):
                    ps = psumpool.tile([128, C], f32, tag="ps", name="ps")
                    for ky in range(5):
                        lhsT = at_all[:, at_off + (r * 5 + ky):
                                      at_off + W_OUT * 10:10]  # [68, 128]
                        rhs = feat_sb[:, (j + ky) * C:(j + ky + 1) * C]
                        if mm_dtype_name != "float32":
                            lhsT = lhsT.bitcast(mm_dt)
                            rhs = rhs.bitcast(mm_dt)
                        nc.tensor.matmul(ps[:], lhsT, rhs,
                                         start=(ky == 0), stop=(ky == 4))
                    ot = ots[(2 * j + r) % 4]
                    nc.vector.tensor_scalar_mul(ot[:], ps[:],
                                                invsum[:, r:r + 1])
                    nc.sync.dma_start(out=out[2 * j + r], in_=ot[:])

    nc.compile()
    return nc


def get_nc(mm_dtype_name="float32"):
    key = mm_dtype_name
    if key not in _NC_CACHE:
        _NC_CACHE[key] = _build_nc(mm_dtype_name)
    return _NC_CACHE[key]


def shard_inputs(features, masks):
    """Full inputs -> per-core input maps (host-side zero padding)."""
    features = np.asarray(features)
    masks = np.asarray(masks)
    in_maps = []
    for c in range(N_CORES):
        b, q = divmod(c, 4)
        y0 = PAIRS * q
        slab = np.zeros((SLAB, W_IN, C), np.float32)
        lo = y0 - 2
        for i in range(SLAB):
            y = lo + i
            if 0 <= y < H_IN:
                slab[i] = features[b, y]
        in_maps.append({
            "feat": np.ascontiguousarray(slab),
            "masks": np.ascontiguousarray(
                masks[b, ROWS_PER_CORE * q:ROWS_PER_CORE * (q + 1)]
            ).astype(np.float32),
        })
    return in_maps


def unshard_outputs(results):
    out = np.empty((B, H_OUT, W_OUT, C), np.float32)
    for c in range(N_CORES):
        b, q = divmod(c, 4)
        out[b, ROWS_PER_CORE * q:ROWS_PER_CORE * (q + 1)] = results[c]["out"]
    return out


def kernel(features, masks):
    from concourse.bass_utils import run_bass_kernel_spmd

    nc = get_nc()
    in_maps = shard_inputs(features, masks)
    res = run_bass_kernel_spmd(nc, in_maps, list(range(N_CORES)))
    return unshard_outputs(res.results)


# revision 16
# speedup vs baseline: 976.0421x; 976.0421x over previous
"""CARAFE upsampling (k=5, x2, C=256) as a Bass/Tile kernel on 8 NeuronCores.

Math (per output pixel):
  out[b, Y, X, c] = sum_{ky,kx} softmax(masks[b,Y,X,:])[ky*5+kx]
                    * feat[b, Y//2+ky-2, X//2+kx-2, c]       (zero padded)

Mapping: pure data parallel over (batch, 32-output-row strips) -> 8 cores.
Each core handles 16 output-row pairs; a row pair (Y=2y, 2y+1) contracts
over x' (64 source cols + 2 zero pad each side -> 68 partitions) with a
banded weight matrix per (row r, ky):
  lhsT[x'pad, X] = exp(mask[Y, X, ky*5 + (x'pad - X//2)])   (band, else 0)
so out_psum[X, c] = sum_ky lhsT_ky.T @ feat_row[y+ky-2]  (PSUM accumulate),
then scaled by 1/sum(exp) on PSUM->SBUF eviction.

The banded lhsT is built by one diagonal SBUF->SBUF DMA per row pair from
an exp'd, freely-reordered weight tile:
  e5[u, kx*20 + v*10 + r*5 + ky] = exp(masks[2y+r, 2u+v, ky*5+kx])
  AT[u+kx (part), (2u+v)*10 + r*5 + ky] = e5[u, ...]
dest AP [[1300,64],[1280,5],[1,20]] / src [[100,64],[20,5],[1,20]]
-> 320 descriptors of 80 contiguous bytes. Out-of-band entries stay zero
(memset once; the in-band region is fully rewritten every iteration), and
band rows touching the x' zero-pad multiply zeroed feature partitions.
"""

import sys

for _p in ("/opt/trn_rl_repo",):
    if _p not in sys.path:
        sys.path.insert(0, _p)

import numpy as np

B = 2
H_IN = 64
W_IN = 64
C = 256
H_OUT = 128
W_OUT = 128
KK = 25
N_CORES = 8
ROWS_PER_CORE = H_OUT * B // N_CORES  # 32 output rows
PAIRS = ROWS_PER_CORE // 2  # 16
SLAB = PAIRS + 4  # feature rows a core touches (16 + 2 pad each side)
XPAD = W_IN + 4  # 68 contraction partitions

_NC_CACHE = {}


def _build_nc(mm_dtype_name="float32", reps=1):
    import concourse.bacc as bacc
    import concourse.mybir as mybir
    from concourse import tile

    dt = mybir.dt
    f32 = dt.float32
    mm_dt = getattr(dt, mm_dtype_name)

    # detect_race_conditions=False: the sim's race shadow over-approximates
    # the diagonal scatter AP's footprint (flat element span read as a
    # per-partition byte span), falsely flagging disjoint tiles. Tile's own
    # dependency tracking handles the diagonal AP correctly (verified with
    # a micro-test: the scatter is sem-ordered against same-tensor
    # memset/producer/consumer accesses).
    nc = bacc.Bacc("TRN2", target_bir_lowering=False, debug=False,
                   num_devices=N_CORES, detect_race_conditions=False)
    feat = nc.dram_tensor("feat", [SLAB, W_IN, C], f32, kind="ExternalInput")
    masks = nc.dram_tensor("masks", [ROWS_PER_CORE, W_OUT, KK], f32,
                           kind="ExternalInput")
    out = nc.dram_tensor("out", [ROWS_PER_CORE, W_OUT, C], f32,
                         kind="ExternalOutput")

    AP = type(feat[:])

    NBUF = 3

    with tile.TileContext(nc) as tc:
        with (
            tc.tile_pool(name="big", bufs=1) as big,
            tc.tile_pool(name="psum", bufs=4, space="PSUM") as psumpool,
        ):
            # All loop tiles are explicit arrays (manual multi-buffering) so
            # that at_all can be created LAST: the sim's coarse span
            # bookkeeping for the diagonal scatter AP extends upward from
            # at_all's base, so nothing may live above it.
            def mk(shape, tag, n):
                return [big.tile(shape, f32, tag=f"{tag}{i}",
                                 name=f"{tag}{i}") for i in range(n)]

            feat_sb = big.tile([XPAD, SLAB * C], f32, tag="feat_sb",
                               name="feat_sb")
            m4s = mk([64, 100], "m4", NBUF)
            e5s = mk([64, 100], "e5", NBUF)
            m2s = mk([128, 50], "m2", NBUF)
            escr = mk([128, KK], "escr", 2)
            esums = mk([128, 2], "esum", NBUF)
            invsums = mk([128, 2], "invsum", NBUF)
            ots = mk([128, C], "ot", 4)
            # merged double-buffered banded-weights tensor, topmost in SBUF
            at_all = big.tile([XPAD, 2 * W_OUT * 10], f32, tag="at_all",
                              name="at_all")
            # DRAM staging for the banded scatter: the SBUF descriptor
            # generators (HWDGE and SWDGE) cannot walk mixed
            # partition+byte ("diagonal") steps, but DRAM is flat, so the
            # diagonal write goes to DRAM and a plain 2D load rebuilds the
            # banded SBUF tile.
            stage = nc.dram_tensor("at_stage", [2, XPAD, W_OUT * 10], f32,
                                   kind="Internal")

            nc.vector.memset(feat_sb[:], 0.0)
            nc.vector.memset(at_all[:], 0.0)
            # zero both staging buffers once (bands are fully rewritten
            # every iteration; out-of-band stays zero forever)
            for hb in range(2):
                nc.sync.dma_start(out=stage[hb], in_=at_all[:, :W_OUT * 10])

            # Feature rows: partition x'pad = x' + 2 (0,1,66,67 stay zero).
            nc.sync.dma_start(
                out=feat_sb[2:2 + W_IN, :].rearrange("x (r c) -> x r c",
                                                     r=SLAB),
                in_=feat[:].rearrange("r x c -> x r c"),
            )

            for j in range(PAIRS * reps):
                rep, j = divmod(j, PAIRS)
                at_off = (j % 2) * W_OUT * 10

                # masks for rows (2j, 2j+1): m4[u, r*50 + v*25 + p]
                m4 = m4s[j % NBUF]
                nc.sync.dma_start(
                    out=m4[:].rearrange("u (r v p) -> u r (v p)", r=2, v=2),
                    in_=masks[2 * j:2 * j + 2].rearrange(
                        "r (u v) p -> u r (v p)", v=2),
                )

                # exp into scatter-ready layout
                # e5[u, kx*20 + v*10 + r*5 + ky] = exp(m4[u, r*50+v*25+ky*5+kx])
                e5 = e5s[j % NBUF]
                m4v = m4[:].rearrange("u (r v ky kx) -> u r v kx ky",
                                      r=2, v=2, ky=5)
                e5v = e5[:].rearrange("u (kx v r ky) -> u r v kx ky",
                                      kx=5, v=2, r=2)
                for r in range(2):
                    for v in range(2):
                        nc.scalar.activation(
                            out=e5v[:, r, v],
                            in_=m4v[:, r, v],
                            func=mybir.ActivationFunctionType.Exp,
                        )

                # Softmax denominators in output-pixel layout: second load of
                # the same mask rows as [X=128, (r, p)], exp with accum_out
                # giving per-partition sums, then reciprocal.
                m2 = m2s[j % NBUF]
                nc.sync.dma_start(
                    out=m2[:].rearrange("x (r p) -> x r p", r=2),
                    in_=masks[2 * j:2 * j + 2].rearrange("r x p -> x r p"),
                )
                esum = esums[j % NBUF]
                for r in range(2):
                    nc.scalar.activation(
                        out=escr[r][:],
                        in_=m2[:, r * KK:(r + 1) * KK],
                        func=mybir.ActivationFunctionType.Exp,
                        accum_out=esum[:, r:r + 1],
                    )
                invsum = invsums[j % NBUF]
                nc.vector.reciprocal(invsum[:], esum[:])

                # Banded scatter via DRAM staging:
                #   stage[hb][u+kx, (2u+v)*10 + r*5 + ky] = e5[u, ...]
                hb = j % 2
                W10 = W_OUT * 10
                stage_full = stage[hb]
                dst = AP(tensor=stage_full.tensor,
                         offset=stage_full.offset,
                         ap=[[W10 + 20, 64], [W10, 5], [1, 20]])
                e5_full = e5[:]
                src = AP(tensor=e5_full.tensor, offset=e5_full.offset,
                         ap=[[100, 64], [20, 5], [1, 20]])
                nc.sync.dma_start(out=dst, in_=src)
                # load the banded tile back into SBUF
                nc.sync.dma_start(out=at_all[:, at_off:at_off + W10],
                                  in_=stage[hb])

                # 5 accumulating matmuls per output row
                for r in range(2):
                    ps = psumpool.tile([128, C], f32, tag="ps", name="ps")
                    for ky in range(5):
                        lhsT = at_all[:, at_off + (r * 5 + ky):
                                      at_off + W_OUT * 10:10]  # [68, 128]
                        rhs = feat_sb[:, (j + ky) * C:(j + ky + 1) * C]
                        if mm_dtype_name != "float32":
                            lhsT = lhsT.bitcast(mm_dt)
                            rhs = rhs.bitcast(mm_dt)
                        nc.tensor.matmul(ps[:], lhsT, rhs,
                                         start=(ky == 0), stop=(ky == 4))
                    ot = ots[(2 * j + r) % 4]
                    nc.vector.tensor_scalar_mul(ot[:], ps[:],
                                                invsum[:, r:r + 1])
                    nc.sync.dma_start(out=out[2 * j + r], in_=ot[:])

    nc.compile()
    return nc


def get_nc(mm_dtype_name="float32", reps=1):
    key = (mm_dtype_name, reps)
    if key not in _NC_CACHE:
        _NC_CACHE[key] = _build_nc(mm_dtype_name, reps)
    return _NC_CACHE[key]


def shard_inputs(features, masks):
    """Full inputs -> per-core input maps (host-side zero padding)."""
    features = np.asarray(features)
    masks = np.asarray(masks)
    in_maps = []
    for c in range(N_CORES):
        b, q = divmod(c, 4)
        y0 = PAIRS * q
        slab = np.zeros((SLAB, W_IN, C), np.float32)
        lo = y0 - 2
        for i in range(SLAB):
            y = lo + i
            if 0 <= y < H_IN:
                slab[i] = features[b, y]
        in_maps.append({
            "feat": np.ascontiguousarray(slab),
            "masks": np.ascontiguousarray(
                masks[b, ROWS_PER_CORE * q:ROWS_PER_CORE * (q + 1)]
            ).astype(np.float32),
        })
    return in_maps


def unshard_outputs(results):
    out = np.empty((B, H_OUT, W_OUT, C), np.float32)
    for c in range(N_CORES):
        b, q = divmod(c, 4)
        out[b, ROWS_PER_CORE * q:ROWS_PER_CORE * (q + 1)] = results[c]["out"]
    return out


def kernel(features, masks):
    from concourse.bass_utils import run_bass_kernel_spmd

    nc = get_nc()
    in_maps = shard_inputs(features, masks)
    res = run_bass_kernel_spmd(nc, in_maps, list(range(N_CORES)))
    return unshard_outputs(res.results)
